# revision 11
# baseline (speedup 1.0000x reference)
"""AttentiveTransformer (fc -> BatchNorm(batch stats) -> *prior -> sparsemax) on 8 trn2 cores.

Data-parallel over the batch dim. Per core:
  phase 1: stream x, accumulate x^T x (4 parallel PSUM chains) and sum(x)
    (2 chains) on PE, transpose x into a persistent SBUF xT.
  allreduce the [128,129] stats pack, derive the BN scale, center xT by the
    batch mean (zn = (x - xbar) @ (s*W)^T + beta: the fc bias and BN mean
    cancel exactly).
  phase 2 per 1024-row superblock: z = xT_c @ W2T (PSUM), pb = z*prior,
    top-8 per row -> tau8 = max_k (cumsum_k - 1)/k  (a guaranteed Michelot
    start: active(tau8) always contains the sparsemax support), then two
    Michelot steps with fused accumulation passes:
      S0 = sum pb*[pb>tau8] (gpsimd), N0 = #[pb>tau8] (DVE)  -> theta1
      f1 = sum relu(pb-theta1) (ACT), N1 = #[pb>theta1] (DVE) -> tau
    (iteration 1 is exact for 99.97% of rows; iteration 2 covers the rest;
     converged rows are fixed points so extra steps are harmless),
    sm = relu(pb - tau) (ACT), new_prior = prior*sm (gpsimd).
"""

import numpy as np

import concourse.bass as bass
import concourse.bacc as bacc
import concourse.mybir as mybir
from concourse.tile import TileContext
from concourse.masks import make_identity
from concourse.bass_utils import run_bass_kernel_spmd

f32 = mybir.dt.float32
A = mybir.AluOpType
AF = mybir.ActivationFunctionType

B_FULL = 262144
NA = 128
D = 256
NCORES = 8
EPS = 1e-5

CHUNK = 2048          # phase-1 rows per DMA (1 MiB)
TPC = CHUNK // 128    # 16 sub-tiles per chunk
SBROWS = 1024         # phase-2 rows per superblock
TSB = SBROWS // 128   # 8 sub-tiles per superblock
NXTX = 4              # parallel xtx accumulation chains
NXS = 2               # parallel xsum accumulation chains


def build_kernel(BS: int, B_total: int, beta_zero: bool) -> bass.Bass:
    assert BS % CHUNK == 0
    nchunk = BS // CHUNK
    nsb = BS // SBROWS

    nc = bacc.Bacc(None, num_devices=NCORES)
    xd = nc.dram_tensor("xsh", [BS, NA], f32, kind="ExternalInput")
    pd = nc.dram_tensor("psh", [BS, D], f32, kind="ExternalInput")
    Wd = nc.dram_tensor("W", [D, NA], f32, kind="ExternalInput")
    gd = nc.dram_tensor("gvec", [1, D], f32, kind="ExternalInput")
    ed = nc.dram_tensor("evec", [1, D], f32, kind="ExternalInput")
    smd = nc.dram_tensor("smo", [BS, D], f32, kind="ExternalOutput")
    npd = nc.dram_tensor("npo", [BS, D], f32, kind="ExternalOutput")

    with TileContext(nc) as tc:
        with (
            tc.tile_pool(name="big", bufs=1) as big,
            tc.tile_pool(name="consts", bufs=1) as consts,
            tc.tile_pool(name="dram", bufs=1, space="DRAM") as dram,
        ):
            xT = big.tile([128, BS], f32)

            ident = consts.tile([128, 128], f32)
            make_identity(nc, ident[:, :])
            ones_col = consts.tile([128, 1], f32)
            nc.vector.memset(ones_col[:, :], 1.0)
            ones_row = consts.tile([1, 128], f32)
            nc.vector.memset(ones_row[:, :], 1.0)
            # scan mask: 0 at the start of each 8-group (resets the running
            # cumsum at sub-tile boundaries); invk[k] = 1/(k+1)
            smask = consts.tile([128, TSB, 8], f32)
            nc.vector.memset(smask[:, :, :], 1.0)
            nc.vector.memset(smask[:, :, 0], 0.0)
            invk = consts.tile([128, TSB, 8], f32)
            for k in range(8):
                nc.vector.memset(invk[:, :, k], 1.0 / (k + 1))

            Wt0 = consts.tile([128, NA], f32)
            Wt1 = consts.tile([128, NA], f32)
            nc.sync.dma_start(out=Wt0[:, :], in_=Wd[0:128, :])
            nc.sync.dma_start(out=Wt1[:, :], in_=Wd[128:256, :])
            gv = consts.tile([1, D], f32)
            nc.sync.dma_start(out=gv[:, :], in_=gd[:, :])
            ev = consts.tile([1, D], f32)
            nc.sync.dma_start(out=ev[:, :], in_=ed[:, :])

            WT = consts.tile([128, D], f32)
            stats = consts.tile([128, 129], f32)

            with (
                tc.tile_pool(name="p1", bufs=3) as p1pool,
                tc.tile_pool(name="ps1", bufs=1, space="PSUM") as ps1,
                tc.tile_pool(name="ps1t", bufs=2, space="PSUM") as ps1t,
            ):
                tpW0 = ps1t.tile([128, 128], f32, tag="tp")
                nc.tensor.transpose(tpW0[:, :], Wt0[:, :], ident[:, :])
                nc.vector.tensor_copy(out=WT[:, 0:128], in_=tpW0[:, :])
                tpW1 = ps1t.tile([128, 128], f32, tag="tp")
                nc.tensor.transpose(tpW1[:, :], Wt1[:, :], ident[:, :])
                nc.vector.tensor_copy(out=WT[:, 128:256], in_=tpW1[:, :])

                xtxp = [ps1.tile([128, 128], f32, tag=f"xtx{i}", name=f"xtx{i}") for i in range(NXTX)]
                xsump = [ps1.tile([128, 1], f32, tag=f"xsum{i}", name=f"xsum{i}") for i in range(NXS)]
                ntile = nchunk * TPC
                for c in range(nchunk):
                    xin = p1pool.tile([128, TPC, NA], f32, tag="xin")
                    nc.sync.dma_start(
                        out=xin[:, :, :],
                        in_=xd[c * CHUNK : (c + 1) * CHUNK, :].rearrange(
                            "(p t) n -> p t n", p=128
                        ),
                    )
                    for t in range(TPC):
                        g = c * TPC + t
                        nc.tensor.matmul(
                            xtxp[g % NXTX][:, :], lhsT=xin[:, t, :], rhs=xin[:, t, :],
                            start=(g < NXTX), stop=(g >= ntile - NXTX),
                        )
                        nc.tensor.matmul(
                            xsump[g % NXS][:, :], lhsT=xin[:, t, :], rhs=ones_col[:, :],
                            start=(g < NXS), stop=(g >= ntile - NXS),
                        )
                        tp = ps1t.tile([128, 128], f32, tag="tp")
                        nc.tensor.transpose(tp[:, :], xin[:, t, :], ident[:, :])
                        col = c * CHUNK + t * 128
                        nc.vector.tensor_copy(out=xT[:, col : col + 128], in_=tp[:, :])
                # combine the parallel chains into the stats pack (at most one
                # PSUM operand per TensorTensor op)
                nc.vector.tensor_copy(out=stats[:, 0:128], in_=xtxp[0][:, :])
                for i in range(1, NXTX):
                    nc.vector.tensor_add(stats[:, 0:128], stats[:, 0:128], xtxp[i][:, :])
                nc.vector.tensor_copy(out=stats[:, 128:129], in_=xsump[0][:, :])
                for i in range(1, NXS):
                    nc.vector.tensor_add(
                        stats[:, 128:129], stats[:, 128:129], xsump[i][:, :]
                    )

            # ---- cross-core stats allreduce ----
            cc_in = dram.tile([128, 129], f32)
            cc_out = dram.tile([128, 129], f32)
            nc.sync.dma_start(out=cc_in[:, :], in_=stats[:, :])
            nc.gpsimd.collective_compute(
                "AllReduce",
                A.add,
                replica_groups=[list(range(NCORES))],
                ins=[cc_in[:, :].opt()],
                outs=[cc_out[:, :].opt()],
            )
            gstats = consts.tile([128, 129], f32)
            nc.sync.dma_start(out=gstats[:, :], in_=cc_out[:, :])

            # ---- BN stats -> scale vector + x centering ----
            xbarT = consts.tile([128, 1], f32)
            nc.vector.tensor_scalar(
                out=xbarT[:, :], in0=gstats[:, 128:129],
                scalar1=1.0 / B_total, scalar2=None, op0=A.mult,
            )
            for c in range(nchunk):
                sl = xT[:, c * CHUNK : (c + 1) * CHUNK]
                nc.vector.tensor_scalar(
                    out=sl, in0=sl, scalar1=xbarT[:, 0:1], scalar2=None,
                    op0=A.subtract,
                )

            with tc.tile_pool(name="ps2", bufs=1, space="PSUM") as ps2:
                xbrp = ps2.tile([1, 128], f32, tag="xbr")
                nc.tensor.transpose(xbrp[:, :], xbarT[:, :], ident[:, :])
                xbar_row = consts.tile([1, 128], f32)
                nc.vector.tensor_copy(out=xbar_row[:, :], in_=xbrp[:, :])

                outerp = ps2.tile([128, 128], f32, tag="outer")
                nc.tensor.matmul(
                    outerp[:, :], lhsT=xbar_row[:, :], rhs=xbar_row[:, :],
                    start=True, stop=True,
                )
                Cm = consts.tile([128, 128], f32)
                # C = xtx/B - xbar xbar^T
                nc.vector.scalar_tensor_tensor(
                    out=Cm[:, :], in0=gstats[:, 0:128], scalar=1.0 / B_total,
                    in1=outerp[:, :], op0=A.mult, op1=A.subtract,
                )
                CWp = ps2.tile([128, D], f32, tag="cw")
                nc.tensor.matmul(
                    CWp[:, :], lhsT=Cm[:, :], rhs=WT[:, :], start=True, stop=True
                )
                prod = consts.tile([128, D], f32)
                nc.vector.tensor_mul(prod[:, :], WT[:, :], CWp[:, :])
                varp = ps2.tile([1, D], f32, tag="var")
                nc.tensor.matmul(
                    varp[:, :], lhsT=ones_col[:, :], rhs=prod[:, :],
                    start=True, stop=True,
                )
                vtmp = consts.tile([1, D], f32)
                nc.vector.tensor_scalar(
                    out=vtmp[:, :], in0=varp[:, :], scalar1=EPS, scalar2=None,
                    op0=A.add,
                )
                vrec = consts.tile([1, D], f32)
                nc.vector.reciprocal(vrec[:, :], vtmp[:, :])
                invstd = consts.tile([1, D], f32)
                nc.scalar.sqrt(invstd[:, :], vrec[:, :])
                svec = consts.tile([1, D], f32)
                nc.vector.tensor_mul(svec[:, :], gv[:, :], invstd[:, :])

                sbp = ps2.tile([128, D], f32, tag="sb")
                nc.tensor.matmul(
                    sbp[:, :], lhsT=ones_row[:, :], rhs=svec[:, :],
                    start=True, stop=True,
                )
                W2T = consts.tile([128, D], f32)
                nc.vector.tensor_mul(W2T[:, :], WT[:, :], sbp[:, :])

                beta_b = None
                if not beta_zero:
                    bbp = ps2.tile([128, D], f32, tag="bb")
                    nc.tensor.matmul(
                        bbp[:, :], lhsT=ones_row[:, :], rhs=ev[:, :],
                        start=True, stop=True,
                    )
                    beta_b = consts.tile([128, D], f32)
                    nc.vector.tensor_copy(out=beta_b[:, :], in_=bbp[:, :])

            # ---- phase 2 ----
            with (
                tc.tile_pool(name="p2", bufs=2) as p2,
                tc.tile_pool(name="p2s", bufs=3) as p2s,
                tc.tile_pool(name="psz", bufs=2, space="PSUM") as psz,
            ):
                for sb in range(nsb):
                    c, h = sb // 2, sb % 2
                    base = c * CHUNK
                    toff = h * TSB
                    prv = pd[base : base + CHUNK, :].rearrange("(p t) d -> p t d", p=128)
                    pr = p2.tile([128, TSB, D], f32, tag="pr")
                    nc.sync.dma_start(out=pr[:, :, :], in_=prv[:, toff : toff + TSB, :])

                    zp = psz.tile([128, TSB, D], f32, tag="z")
                    for t in range(TSB):
                        col = base + (toff + t) * 128
                        nc.tensor.matmul(
                            zp[:, t, :], lhsT=xT[:, col : col + 128], rhs=W2T[:, :],
                            start=True, stop=True,
                        )
                    # z out of PSUM on ACT, then pb = z*prior in place on gpsimd
                    pb = p2.tile([128, TSB, D], f32, tag="pb")
                    if beta_zero:
                        nc.scalar.copy(out=pb[:, :, :], in_=zp[:, :, :])
                    else:
                        bview = beta_b[:, :].rearrange("p (o d) -> p o d", o=1)
                        bview = bview.to_broadcast([128, TSB, D])
                        nc.vector.tensor_add(pb[:, :, :], zp[:, :, :], bview)
                    nc.gpsimd.tensor_mul(pb[:, :, :], pb[:, :, :], pr[:, :, :])

                    # top-8 -> tau8 = max_{k<=8} (cs_k - 1)/k
                    v = p2s.tile([128, TSB, 8], f32, tag="v")
                    for t in range(TSB):
                        nc.vector.max(out=v[:, t, :], in_=pb[:, t, :])
                    cs = p2s.tile([128, TSB, 8], f32, tag="cs")
                    nc.vector.tensor_tensor_scan(
                        out=cs[:, :, :].rearrange("p a b -> p (a b)"),
                        data0=smask[:, :, :].rearrange("p a b -> p (a b)"),
                        data1=v[:, :, :].rearrange("p a b -> p (a b)"),
                        initial=0.0,
                        op0=A.mult,
                        op1=A.add,
                    )
                    tv = p2s.tile([128, TSB, 8], f32, tag="tv")
                    nc.vector.scalar_tensor_tensor(
                        out=tv[:, :, :].rearrange("p a b -> p (a b)"),
                        in0=cs[:, :, :].rearrange("p a b -> p (a b)"),
                        scalar=-1.0,
                        in1=invk[:, :, :].rearrange("p a b -> p (a b)"),
                        op0=A.add,
                        op1=A.mult,
                    )
                    tau8 = p2s.tile([128, TSB], f32, tag="tau8")
                    nc.vector.tensor_reduce(
                        out=tau8[:, :], in_=tv[:, :, :], axis=mybir.AxisListType.X,
                        op=A.max,
                    )

                    # Michelot iteration 1 at theta0 = tau8:
                    #   S0 = sum pb*[pb>tau8], N0 = #[pb>tau8] (both DVE)
                    scr = p2.tile([128, TSB, D], f32, tag="scr")
                    S0 = p2s.tile([128, TSB], f32, tag="S0")
                    N0 = p2s.tile([128, TSB], f32, tag="N0")
                    for t in range(TSB):
                        nc.vector.scalar_tensor_tensor(
                            out=scr[:, t, :], in0=pb[:, t, :],
                            scalar=tau8[:, t : t + 1], in1=pb[:, t, :],
                            op0=A.is_gt, op1=A.mult,
                            accum_out=S0[:, t : t + 1],
                        )
                    for t in range(TSB):
                        nc.vector.tensor_scalar(
                            out=scr[:, t, :], in0=pb[:, t, :],
                            scalar1=tau8[:, t : t + 1], scalar2=None, op0=A.is_gt,
                            op1=A.add, accum_out=N0[:, t : t + 1],
                        )
                    rN0 = p2s.tile([128, TSB], f32, tag="rN0")
                    nc.vector.reciprocal(rN0[:, :], N0[:, :])
                    th1 = p2s.tile([128, TSB], f32, tag="th1")
                    nc.vector.scalar_tensor_tensor(
                        out=th1[:, :], in0=S0[:, :], scalar=-1.0, in1=rN0[:, :],
                        op0=A.add, op1=A.mult,
                    )
                    nth1 = p2s.tile([128, TSB], f32, tag="nth1")
                    nc.vector.tensor_scalar(
                        out=nth1[:, :], in0=th1[:, :], scalar1=-1.0, scalar2=None,
                        op0=A.mult,
                    )

                    # Michelot iteration 2 at theta1:
                    #   f1 = sum relu(pb-theta1) (ACT), N1 = #[pb>theta1] (DVE)
                    f1 = p2s.tile([128, TSB], f32, tag="f1")
                    N1 = p2s.tile([128, TSB], f32, tag="N1")
                    for t in range(TSB):
                        nc.scalar.activation(
                            out=scr[:, t, :], in_=pb[:, t, :], func=AF.Relu,
                            bias=nth1[:, t : t + 1], scale=1.0,
                            accum_out=f1[:, t : t + 1],
                        )
                    for t in range(TSB):
                        nc.vector.tensor_scalar(
                            out=scr[:, t, :], in0=pb[:, t, :],
                            scalar1=th1[:, t : t + 1], scalar2=None, op0=A.is_gt,
                            op1=A.add, accum_out=N1[:, t : t + 1],
                        )
                    rN1 = p2s.tile([128, TSB], f32, tag="rN1")
                    nc.vector.reciprocal(rN1[:, :], N1[:, :])
                    dt1 = p2s.tile([128, TSB], f32, tag="dt1")
                    nc.vector.scalar_tensor_tensor(
                        out=dt1[:, :], in0=f1[:, :], scalar=-1.0, in1=rN1[:, :],
                        op0=A.add, op1=A.mult,
                    )
                    # ntau = -(theta1 + dt1)
                    ntau = p2s.tile([128, TSB], f32, tag="ntau")
                    nc.vector.scalar_tensor_tensor(
                        out=ntau[:, :], in0=th1[:, :], scalar=-1.0, in1=dt1[:, :],
                        op0=A.mult, op1=A.subtract,
                    )

                    for t in range(TSB):
                        nc.scalar.activation(
                            out=scr[:, t, :], in_=pb[:, t, :], func=AF.Relu,
                            bias=ntau[:, t : t + 1], scale=1.0,
                        )
                    nc.gpsimd.tensor_mul(pr[:, :, :], scr[:, :, :], pr[:, :, :])

                    smv = smd[base : base + CHUNK, :].rearrange("(p t) d -> p t d", p=128)
                    npv = npd[base : base + CHUNK, :].rearrange("(p t) d -> p t d", p=128)
                    nc.sync.dma_start(out=smv[:, toff : toff + TSB, :], in_=scr[:, :, :])
                    nc.sync.dma_start(out=npv[:, toff : toff + TSB, :], in_=pr[:, :, :])
    nc.compile()
    return nc


_CACHE: dict = {}


def _get_kernel(BS: int, B_total: int, beta_zero: bool) -> bass.Bass:
    key = (BS, B_total, beta_zero)
    if key not in _CACHE:
        _CACHE[key] = build_kernel(BS, B_total, beta_zero)
    return _CACHE[key]


def kernel(x, prior_scales, W, b, gamma, beta):
    x = np.ascontiguousarray(np.asarray(x, dtype=np.float32))
    prior_scales = np.ascontiguousarray(np.asarray(prior_scales, dtype=np.float32))
    W = np.ascontiguousarray(np.asarray(W, dtype=np.float32))
    gamma = np.asarray(gamma, dtype=np.float32).reshape(1, -1)
    beta = np.asarray(beta, dtype=np.float32).reshape(1, -1)
    # the fc bias b cancels exactly in training-mode batchnorm (z - mean(z));
    # beta is handled on-device (fast path when all-zero).
    assert x.shape[1] == NA and W.shape == (D, NA)
    B = x.shape[0]
    assert B % (NCORES * CHUNK) == 0
    BS = B // NCORES
    beta_zero = not np.any(beta)

    nc = _get_kernel(BS, B, beta_zero)
    in_maps = []
    for i in range(NCORES):
        in_maps.append(
            {
                "xsh": x[i * BS : (i + 1) * BS],
                "psh": prior_scales[i * BS : (i + 1) * BS],
                "W": W,
                "gvec": np.ascontiguousarray(gamma),
                "evec": np.ascontiguousarray(beta),
            }
        )
    res = run_bass_kernel_spmd(nc, in_maps, core_ids=list(range(NCORES)))
    sm = np.concatenate([res.results[i]["smo"] for i in range(NCORES)], axis=0)
    npr = np.concatenate([res.results[i]["npo"] for i in range(NCORES)], axis=0)
    return sm, npr


# revision 12
# speedup vs baseline: 59.4397x; 59.4397x over previous
"""AttentiveTransformer (fc -> BatchNorm(batch stats) -> *prior -> sparsemax) on 8 trn2 cores.

Data-parallel over the batch dim. Per core:
  phase 1: stream x, accumulate x^T x (4 parallel PSUM chains) and sum(x)
    (2 chains) on PE, transpose x into a persistent SBUF xT.
  allreduce the [128,129] stats pack, derive the BN scale, center xT by the
    batch mean (zn = (x - xbar) @ (s*W)^T + beta: the fc bias and BN mean
    cancel exactly).
  phase 2 per 1024-row superblock: z = xT_c @ W2T (PSUM), z out of PSUM on
    ACT, pb = z*prior in place on gpsimd, top-8 per row ->
    tau8 = max_k (cumsum_k - 1)/k (a guaranteed Michelot start: active(tau8)
    always contains the sparsemax support), then two Michelot steps with
    fused accumulation passes:
      S0 = sum pb*[pb>tau8] (DVE stt), N0 = #[pb>tau8] (DVE ts) -> theta1
      f1 = sum relu(pb-theta1) (ACT), N1 = #[pb>theta1] (DVE ts) -> tau
    (iteration 1 is exact for 99.97% of rows; iteration 2 covers the rest;
     converged rows are fixed points so extra steps are harmless),
    sm = relu(pb - tau) (ACT), new_prior = prior*sm (gpsimd).

reps > 1 re-emits the whole computation serially (through shared tiles) for
device-time measurement: T(reps=R) ~ overhead + R*T_oneshot.
"""

import numpy as np

import concourse.bass as bass
import concourse.bacc as bacc
import concourse.mybir as mybir
from concourse.tile import TileContext
from concourse.masks import make_identity
from concourse.bass_utils import run_bass_kernel_spmd

f32 = mybir.dt.float32
A = mybir.AluOpType
AF = mybir.ActivationFunctionType

B_FULL = 262144
NA = 128
D = 256
NCORES = 8
EPS = 1e-5

CHUNK = 2048          # phase-1 rows per DMA (1 MiB)
TPC = CHUNK // 128    # 16 sub-tiles per chunk
SBROWS = 1024         # phase-2 rows per superblock
TSB = SBROWS // 128   # 8 sub-tiles per superblock
NXTX = 4              # parallel xtx accumulation chains
NXS = 2               # parallel xsum accumulation chains


def build_kernel(BS: int, B_total: int, beta_zero: bool, reps: int = 1) -> bass.Bass:
    assert BS % CHUNK == 0
    nchunk = BS // CHUNK
    nsb = BS // SBROWS

    nc = bacc.Bacc(None, num_devices=NCORES)
    xd = nc.dram_tensor("xsh", [BS, NA], f32, kind="ExternalInput")
    pd = nc.dram_tensor("psh", [BS, D], f32, kind="ExternalInput")
    Wd = nc.dram_tensor("W", [D, NA], f32, kind="ExternalInput")
    gd = nc.dram_tensor("gvec", [1, D], f32, kind="ExternalInput")
    ed = nc.dram_tensor("evec", [1, D], f32, kind="ExternalInput")
    smd = nc.dram_tensor("smo", [BS, D], f32, kind="ExternalOutput")
    npd = nc.dram_tensor("npo", [BS, D], f32, kind="ExternalOutput")

    with TileContext(nc) as tc:
        with (
            tc.tile_pool(name="big", bufs=1) as big,
            tc.tile_pool(name="consts", bufs=1) as consts,
            tc.tile_pool(name="dram", bufs=1, space="DRAM") as dram,
        ):
            xT = big.tile([128, BS], f32)

            ident = consts.tile([128, 128], f32)
            make_identity(nc, ident[:, :])
            ones_col = consts.tile([128, 1], f32)
            nc.vector.memset(ones_col[:, :], 1.0)
            ones_row = consts.tile([1, 128], f32)
            nc.vector.memset(ones_row[:, :], 1.0)
            # scan mask: 0 at the start of each 8-group (resets the running
            # cumsum at sub-tile boundaries); invk[k] = 1/(k+1)
            smask = consts.tile([128, TSB, 8], f32)
            nc.vector.memset(smask[:, :, :], 1.0)
            nc.vector.memset(smask[:, :, 0], 0.0)
            invk = consts.tile([128, TSB, 8], f32)
            for k in range(8):
                nc.vector.memset(invk[:, :, k], 1.0 / (k + 1))

            Wt0 = consts.tile([128, NA], f32)
            Wt1 = consts.tile([128, NA], f32)
            nc.sync.dma_start(out=Wt0[:, :], in_=Wd[0:128, :])
            nc.sync.dma_start(out=Wt1[:, :], in_=Wd[128:256, :])
            gv = consts.tile([1, D], f32)
            nc.sync.dma_start(out=gv[:, :], in_=gd[:, :])
            ev = consts.tile([1, D], f32)
            nc.sync.dma_start(out=ev[:, :], in_=ed[:, :])

            WT = consts.tile([128, D], f32)
            stats = consts.tile([128, 129], f32)
            gstats = consts.tile([128, 129], f32)
            xbarT = consts.tile([128, 1], f32)
            xbar_row = consts.tile([1, 128], f32)
            Cm = consts.tile([128, 128], f32)
            prod = consts.tile([128, D], f32)
            vtmp = consts.tile([1, D], f32)
            vrec = consts.tile([1, D], f32)
            invstd = consts.tile([1, D], f32)
            svec = consts.tile([1, D], f32)
            W2T = consts.tile([128, D], f32)
            beta_b = consts.tile([128, D], f32) if not beta_zero else None

            cc_in = dram.tile([128, 129], f32)
            cc_out = dram.tile([128, 129], f32)

            with tc.tile_pool(name="ps0", bufs=2, space="PSUM") as ps0:
                tpW0 = ps0.tile([128, 128], f32, tag="tpw")
                nc.tensor.transpose(tpW0[:, :], Wt0[:, :], ident[:, :])
                nc.vector.tensor_copy(out=WT[:, 0:128], in_=tpW0[:, :])
                tpW1 = ps0.tile([128, 128], f32, tag="tpw")
                nc.tensor.transpose(tpW1[:, :], Wt1[:, :], ident[:, :])
                nc.vector.tensor_copy(out=WT[:, 128:256], in_=tpW1[:, :])

            for rep in range(reps):
                # ---- phase 1 ----
                with (
                    tc.tile_pool(name="p1", bufs=3) as p1pool,
                    tc.tile_pool(name="ps1", bufs=1, space="PSUM") as ps1,
                    tc.tile_pool(name="ps1t", bufs=2, space="PSUM") as ps1t,
                ):
                    xtxp = [
                        ps1.tile([128, 128], f32, tag=f"xtx{i}", name=f"xtx{i}_{rep}")
                        for i in range(NXTX)
                    ]
                    xsump = [
                        ps1.tile([128, 1], f32, tag=f"xsum{i}", name=f"xsum{i}_{rep}")
                        for i in range(NXS)
                    ]
                    ntile = nchunk * TPC
                    for c in range(nchunk):
                        xin = p1pool.tile([128, TPC, NA], f32, tag="xin")
                        nc.sync.dma_start(
                            out=xin[:, :, :],
                            in_=xd[c * CHUNK : (c + 1) * CHUNK, :].rearrange(
                                "(p t) n -> p t n", p=128
                            ),
                        )
                        for t in range(TPC):
                            g = c * TPC + t
                            nc.tensor.matmul(
                                xtxp[g % NXTX][:, :], lhsT=xin[:, t, :],
                                rhs=xin[:, t, :],
                                start=(g < NXTX), stop=(g >= ntile - NXTX),
                            )
                            nc.tensor.matmul(
                                xsump[g % NXS][:, :], lhsT=xin[:, t, :],
                                rhs=ones_col[:, :],
                                start=(g < NXS), stop=(g >= ntile - NXS),
                            )
                            tp = ps1t.tile([128, 128], f32, tag="tp")
                            nc.tensor.transpose(tp[:, :], xin[:, t, :], ident[:, :])
                            col = c * CHUNK + t * 128
                            nc.vector.tensor_copy(
                                out=xT[:, col : col + 128], in_=tp[:, :]
                            )
                    # combine parallel chains into the stats pack (at most one
                    # PSUM operand per TensorTensor op)
                    nc.vector.tensor_copy(out=stats[:, 0:128], in_=xtxp[0][:, :])
                    for i in range(1, NXTX):
                        nc.vector.tensor_add(
                            stats[:, 0:128], stats[:, 0:128], xtxp[i][:, :]
                        )
                    nc.vector.tensor_copy(out=stats[:, 128:129], in_=xsump[0][:, :])
                    for i in range(1, NXS):
                        nc.vector.tensor_add(
                            stats[:, 128:129], stats[:, 128:129], xsump[i][:, :]
                        )

                # ---- cross-core stats allreduce ----
                nc.sync.dma_start(out=cc_in[:, :], in_=stats[:, :])
                nc.gpsimd.collective_compute(
                    "AllReduce",
                    A.add,
                    replica_groups=[list(range(NCORES))],
                    ins=[cc_in[:, :].opt()],
                    outs=[cc_out[:, :].opt()],
                )
                nc.sync.dma_start(out=gstats[:, :], in_=cc_out[:, :])

                # ---- BN stats -> scale vector + x centering ----
                nc.vector.tensor_scalar(
                    out=xbarT[:, :], in0=gstats[:, 128:129],
                    scalar1=1.0 / B_total, scalar2=None, op0=A.mult,
                )
                for c in range(nchunk):
                    sl = xT[:, c * CHUNK : (c + 1) * CHUNK]
                    nc.vector.tensor_scalar(
                        out=sl, in0=sl, scalar1=xbarT[:, 0:1], scalar2=None,
                        op0=A.subtract,
                    )

                with tc.tile_pool(name="ps2", bufs=1, space="PSUM") as ps2:
                    xbrp = ps2.tile([1, 128], f32, tag="xbr")
                    nc.tensor.transpose(xbrp[:, :], xbarT[:, :], ident[:, :])
                    nc.vector.tensor_copy(out=xbar_row[:, :], in_=xbrp[:, :])

                    outerp = ps2.tile([128, 128], f32, tag="outer")
                    nc.tensor.matmul(
                        outerp[:, :], lhsT=xbar_row[:, :], rhs=xbar_row[:, :],
                        start=True, stop=True,
                    )
                    # C = xtx/B - xbar xbar^T
                    nc.vector.scalar_tensor_tensor(
                        out=Cm[:, :], in0=gstats[:, 0:128], scalar=1.0 / B_total,
                        in1=outerp[:, :], op0=A.mult, op1=A.subtract,
                    )
                    CWp = ps2.tile([128, D], f32, tag="cw")
                    nc.tensor.matmul(
                        CWp[:, :], lhsT=Cm[:, :], rhs=WT[:, :], start=True, stop=True
                    )
                    nc.vector.tensor_mul(prod[:, :], WT[:, :], CWp[:, :])
                    varp = ps2.tile([1, D], f32, tag="var")
                    nc.tensor.matmul(
                        varp[:, :], lhsT=ones_col[:, :], rhs=prod[:, :],
                        start=True, stop=True,
                    )
                    nc.vector.tensor_scalar(
                        out=vtmp[:, :], in0=varp[:, :], scalar1=EPS, scalar2=None,
                        op0=A.add,
                    )
                    nc.vector.reciprocal(vrec[:, :], vtmp[:, :])
                    nc.scalar.sqrt(invstd[:, :], vrec[:, :])
                    nc.vector.tensor_mul(svec[:, :], gv[:, :], invstd[:, :])

                    sbp = ps2.tile([128, D], f32, tag="sb")
                    nc.tensor.matmul(
                        sbp[:, :], lhsT=ones_row[:, :], rhs=svec[:, :],
                        start=True, stop=True,
                    )
                    nc.vector.tensor_mul(W2T[:, :], WT[:, :], sbp[:, :])

                    if not beta_zero:
                        bbp = ps2.tile([128, D], f32, tag="bb")
                        nc.tensor.matmul(
                            bbp[:, :], lhsT=ones_row[:, :], rhs=ev[:, :],
                            start=True, stop=True,
                        )
                        nc.vector.tensor_copy(out=beta_b[:, :], in_=bbp[:, :])

                # ---- phase 2 ----
                with (
                    tc.tile_pool(name="p2", bufs=2) as p2,
                    tc.tile_pool(name="p2s", bufs=3) as p2s,
                    tc.tile_pool(name="psz", bufs=2, space="PSUM") as psz,
                ):
                    for sb in range(nsb):
                        c, h = sb // 2, sb % 2
                        base = c * CHUNK
                        toff = h * TSB
                        prv = pd[base : base + CHUNK, :].rearrange(
                            "(p t) d -> p t d", p=128
                        )
                        pr = p2.tile([128, TSB, D], f32, tag="pr")
                        nc.sync.dma_start(
                            out=pr[:, :, :], in_=prv[:, toff : toff + TSB, :]
                        )

                        zp = psz.tile([128, TSB, D], f32, tag="z")
                        for t in range(TSB):
                            col = base + (toff + t) * 128
                            nc.tensor.matmul(
                                zp[:, t, :], lhsT=xT[:, col : col + 128],
                                rhs=W2T[:, :],
                                start=True, stop=True,
                            )
                        # z out of PSUM on ACT, then pb = z*prior in place
                        pb = p2.tile([128, TSB, D], f32, tag="pb")
                        if beta_zero:
                            nc.scalar.copy(out=pb[:, :, :], in_=zp[:, :, :])
                        else:
                            bview = beta_b[:, :].rearrange("p (o d) -> p o d", o=1)
                            bview = bview.to_broadcast([128, TSB, D])
                            nc.vector.tensor_add(pb[:, :, :], zp[:, :, :], bview)
                        nc.gpsimd.tensor_mul(pb[:, :, :], pb[:, :, :], pr[:, :, :])

                        # top-8 -> tau8 = max_{k<=8} (cs_k - 1)/k
                        v = p2s.tile([128, TSB, 8], f32, tag="v")
                        for t in range(TSB):
                            nc.vector.max(out=v[:, t, :], in_=pb[:, t, :])
                        cs = p2s.tile([128, TSB, 8], f32, tag="cs")
                        nc.vector.tensor_tensor_scan(
                            out=cs[:, :, :].rearrange("p a b -> p (a b)"),
                            data0=smask[:, :, :].rearrange("p a b -> p (a b)"),
                            data1=v[:, :, :].rearrange("p a b -> p (a b)"),
                            initial=0.0,
                            op0=A.mult,
                            op1=A.add,
                        )
                        tv = p2s.tile([128, TSB, 8], f32, tag="tv")
                        nc.vector.scalar_tensor_tensor(
                            out=tv[:, :, :].rearrange("p a b -> p (a b)"),
                            in0=cs[:, :, :].rearrange("p a b -> p (a b)"),
                            scalar=-1.0,
                            in1=invk[:, :, :].rearrange("p a b -> p (a b)"),
                            op0=A.add,
                            op1=A.mult,
                        )
                        tau8 = p2s.tile([128, TSB], f32, tag="tau8")
                        nc.vector.tensor_reduce(
                            out=tau8[:, :], in_=tv[:, :, :],
                            axis=mybir.AxisListType.X, op=A.max,
                        )

                        # Michelot iteration 1 at theta0 = tau8:
                        #   S0 = sum pb*[pb>tau8], N0 = #[pb>tau8]
                        scr = p2.tile([128, TSB, D], f32, tag="scr")
                        S0 = p2s.tile([128, TSB], f32, tag="S0")
                        N0 = p2s.tile([128, TSB], f32, tag="N0")
                        for t in range(TSB):
                            nc.vector.scalar_tensor_tensor(
                                out=scr[:, t, :], in0=pb[:, t, :],
                                scalar=tau8[:, t : t + 1], in1=pb[:, t, :],
                                op0=A.is_gt, op1=A.mult,
                                accum_out=S0[:, t : t + 1],
                            )
                        for t in range(TSB):
                            nc.vector.tensor_scalar(
                                out=scr[:, t, :], in0=pb[:, t, :],
                                scalar1=tau8[:, t : t + 1], scalar2=None,
                                op0=A.is_gt, op1=A.add,
                                accum_out=N0[:, t : t + 1],
                            )
                        rN0 = p2s.tile([128, TSB], f32, tag="rN0")
                        nc.vector.reciprocal(rN0[:, :], N0[:, :])
                        th1 = p2s.tile([128, TSB], f32, tag="th1")
                        nc.vector.scalar_tensor_tensor(
                            out=th1[:, :], in0=S0[:, :], scalar=-1.0, in1=rN0[:, :],
                            op0=A.add, op1=A.mult,
                        )
                        nth1 = p2s.tile([128, TSB], f32, tag="nth1")
                        nc.vector.tensor_scalar(
                            out=nth1[:, :], in0=th1[:, :], scalar1=-1.0,
                            scalar2=None, op0=A.mult,
                        )

                        # Michelot iteration 2 at theta1:
                        #   f1 = sum relu(pb-theta1) (ACT), N1 = #[pb>theta1]
                        f1 = p2s.tile([128, TSB], f32, tag="f1")
                        N1 = p2s.tile([128, TSB], f32, tag="N1")
                        for t in range(TSB):
                            nc.scalar.activation(
                                out=scr[:, t, :], in_=pb[:, t, :], func=AF.Relu,
                                bias=nth1[:, t : t + 1], scale=1.0,
                                accum_out=f1[:, t : t + 1],
                            )
                        for t in range(TSB):
                            nc.vector.tensor_scalar(
                                out=scr[:, t, :], in0=pb[:, t, :],
                                scalar1=th1[:, t : t + 1], scalar2=None,
                                op0=A.is_gt, op1=A.add,
                                accum_out=N1[:, t : t + 1],
                            )
                        rN1 = p2s.tile([128, TSB], f32, tag="rN1")
                        nc.vector.reciprocal(rN1[:, :], N1[:, :])
                        dt1 = p2s.tile([128, TSB], f32, tag="dt1")
                        nc.vector.scalar_tensor_tensor(
                            out=dt1[:, :], in0=f1[:, :], scalar=-1.0, in1=rN1[:, :],
                            op0=A.add, op1=A.mult,
                        )
                        # ntau = -(theta1 + dt1)
                        ntau = p2s.tile([128, TSB], f32, tag="ntau")
                        nc.vector.scalar_tensor_tensor(
                            out=ntau[:, :], in0=th1[:, :], scalar=-1.0,
                            in1=dt1[:, :], op0=A.mult, op1=A.subtract,
                        )

                        for t in range(TSB):
                            nc.scalar.activation(
                                out=scr[:, t, :], in_=pb[:, t, :], func=AF.Relu,
                                bias=ntau[:, t : t + 1], scale=1.0,
                            )
                        nc.gpsimd.tensor_mul(pr[:, :, :], scr[:, :, :], pr[:, :, :])

                        smv = smd[base : base + CHUNK, :].rearrange(
                            "(p t) d -> p t d", p=128
                        )
                        npv = npd[base : base + CHUNK, :].rearrange(
                            "(p t) d -> p t d", p=128
                        )
                        nc.sync.dma_start(
                            out=smv[:, toff : toff + TSB, :], in_=scr[:, :, :]
                        )
                        nc.sync.dma_start(
                            out=npv[:, toff : toff + TSB, :], in_=pr[:, :, :]
                        )
    nc.compile()
    return nc


_CACHE: dict = {}


def _get_kernel(BS: int, B_total: int, beta_zero: bool, reps: int = 1) -> bass.Bass:
    key = (BS, B_total, beta_zero, reps)
    if key not in _CACHE:
        _CACHE[key] = build_kernel(BS, B_total, beta_zero, reps)
    return _CACHE[key]


def kernel(x, prior_scales, W, b, gamma, beta):
    x = np.ascontiguousarray(np.asarray(x, dtype=np.float32))
    prior_scales = np.ascontiguousarray(np.asarray(prior_scales, dtype=np.float32))
    W = np.ascontiguousarray(np.asarray(W, dtype=np.float32))
    gamma = np.asarray(gamma, dtype=np.float32).reshape(1, -1)
    beta = np.asarray(beta, dtype=np.float32).reshape(1, -1)
    # the fc bias b cancels exactly in training-mode batchnorm (z - mean(z));
    # beta is handled on-device (fast path when all-zero).
    assert x.shape[1] == NA and W.shape == (D, NA)
    B = x.shape[0]
    assert B % (NCORES * CHUNK) == 0
    BS = B // NCORES
    beta_zero = not np.any(beta)

    nc = _get_kernel(BS, B, beta_zero)
    in_maps = []
    for i in range(NCORES):
        in_maps.append(
            {
                "xsh": x[i * BS : (i + 1) * BS],
                "psh": prior_scales[i * BS : (i + 1) * BS],
                "W": W,
                "gvec": np.ascontiguousarray(gamma),
                "evec": np.ascontiguousarray(beta),
            }
        )
    res = run_bass_kernel_spmd(nc, in_maps, core_ids=list(range(NCORES)))
    sm = np.concatenate([res.results[i]["smo"] for i in range(NCORES)], axis=0)
    npr = np.concatenate([res.results[i]["npo"] for i in range(NCORES)], axis=0)
    return sm, npr
